# revision 17
# baseline (speedup 1.0000x reference)
"""Binarized linear kernel for Trainium2, 8 NeuronCores.

Computes out = sign(x) @ sign(W)^T * alpha + bias  for
x [4, 2048, 4096] f32, W [4096, 4096] f32, bias/alpha [4096] f32.

Sharding: R token-shards x C dout-shards = 8 cores (no collectives;
every core owns a disjoint output block).  The host pre-transposes the
x / W shards so the contraction dim (din) lands on SBUF partitions and
stores them as bf16 (sign-preserving for N(0,1) data: bf16 round-to-
nearest only flushes |v| < 2^-133, which never occurs), tiled so every
DMA is a fully contiguous read.

On device: sign -> fp8e4 {-1,0,+1} (exact), matmul in fp8 DoubleRow
mode (K=256 per pass) accumulating fp32 in PSUM (exact), then the
alpha/bias epilogue in fp32.  The result is bit-exact vs the fp32
reference.
"""

from contextlib import ExitStack

import numpy as np

import concourse.bass as bass
import concourse.mybir as mybir
import concourse.tile as tile
from concourse import bacc
from concourse.bass import ts

B, S, DIN, DOUT_FULL = 4, 2048, 4096, 4096
NTOK = B * S

# sharding grid: R token shards x C dout shards
R, C = 2, 4
TOK = NTOK // R
DOUT = DOUT_FULL // C

USE_FP8 = True

# stash of the last BassKernelResults (for test.py to read profile info)
LAST_RESULTS = None


def build_nc(din=DIN, tok=TOK, dout=DOUT, use_fp8=USE_FP8):
    """One NeuronCore program: out[tok, dout] = sign(xT).T @ sign(wT) * alpha + bias.

    Inputs (per core):
      xt [tok//128, 128, din//128, 128] bf16 : x shard, transposed +
          tiled (partition dim second) so each token-tile is contiguous
      wt [din, dout] bf16 : W shard, transposed
      al [dout] f32, bi [dout] f32
    Output: out [tok, dout] f32
    """
    f32 = mybir.dt.float32
    bf16 = mybir.dt.bfloat16
    mm_dt = mybir.dt.float8e4 if use_fp8 else bf16
    nc = bacc.Bacc("TRN2", target_bir_lowering=False)

    P = 128
    KT = din // P          # contraction tiles
    TT = tok // P          # token tiles
    NB = dout // 512       # psum banks per output row-tile

    xt = nc.declare_dram_parameter("xt", [TT, P, KT, P], bf16, isOutput=False)
    wt = nc.declare_dram_parameter("wt", [din, dout], bf16, isOutput=False)
    al = nc.declare_dram_parameter("al", [dout], f32, isOutput=False)
    bi = nc.declare_dram_parameter("bi", [dout], f32, isOutput=False)
    out = nc.declare_dram_parameter("out", [tok, dout], f32, isOutput=True)

    with ExitStack() as ctx:
        tc = ctx.enter_context(tile.TileContext(nc))
        consts = ctx.enter_context(tc.tile_pool(name="consts", bufs=1))
        wpool = ctx.enter_context(tc.tile_pool(name="wpool", bufs=1))
        wstage = ctx.enter_context(tc.tile_pool(name="wstage", bufs=3))
        wtmp = ctx.enter_context(tc.tile_pool(name="wtmp", bufs=2))
        xstage = ctx.enter_context(tc.tile_pool(name="xstage", bufs=5))
        xpool = ctx.enter_context(tc.tile_pool(name="xpool", bufs=6))
        opool = ctx.enter_context(tc.tile_pool(name="opool", bufs=4))
        pspool = ctx.enter_context(tc.tile_pool(name="psum", bufs=4, space="PSUM"))

        # broadcast alpha/bias along partitions once: [128, dout]
        alphaB = consts.tile([P, dout], f32)
        biasB = consts.tile([P, dout], f32)
        a_ap = al[:]
        nc.gpsimd.dma_start(
            out=alphaB,
            in_=bass.AP(tensor=a_ap.tensor, offset=a_ap.offset,
                        ap=[[0, P]] + list(a_ap.ap)),
        )
        b_ap = bi[:]
        nc.gpsimd.dma_start(
            out=biasB,
            in_=bass.AP(tensor=b_ap.tensor, offset=b_ap.offset,
                        ap=[[0, P]] + list(b_ap.ap)),
        )

        # token-tile 0 first so its load/sign overlaps the W stream; sign
        # in chunks so the first matmul only waits for the first k-pair
        xstg0 = xstage.tile([P, KT, P], bf16, tag="xstg")
        nc.sync.dma_start(out=xstg0, in_=xt[0])
        xb0 = xpool.tile([P, KT, P], mm_dt, tag="xb")
        x0_chunks = [(0, 2), (2, 6), (8, 12), (20, KT - 20)] if KT >= 32 else [(0, KT)]
        for c0, cn in x0_chunks:
            nc.scalar.sign(xb0[:, c0:c0 + cn, :], xstg0[:, c0:c0 + cn, :])

        # weights: stream bf16 in groups of din-tiles (small leading groups
        # so PE starts early), sign -> mm_dt, keep resident [128, KT, dout]
        w_groups = [2, 2, 2, 2] + [4] * ((KT - 8) // 4) if KT >= 16 else [2] * (KT // 2)
        assert sum(w_groups) == KT
        wsb = wpool.tile([P, KT, dout], mm_dt)
        k0 = 0
        for gi, wg in enumerate(w_groups):
            wchunk = wstage.tile([P, 4, dout], bf16)
            nc.sync.dma_start(
                out=wchunk[:, :wg, :],
                in_=wt[k0 * P:(k0 + wg) * P, :].rearrange("(g p) d -> p g d", p=P))
            if gi % 2 == 1:
                nc.scalar.sign(wsb[:, k0:k0 + wg, :], wchunk[:, :wg, :])
            else:
                # exact sign on DVE: (v>0) - (v<0), keeps ACT free for x
                b1 = wtmp.tile([P, 4, dout], bf16)
                nc.vector.tensor_scalar(
                    b1[:, :wg, :], wchunk[:, :wg, :], 0.0, None,
                    mybir.AluOpType.is_lt)
                nc.vector.scalar_tensor_tensor(
                    wsb[:, k0:k0 + wg, :], wchunk[:, :wg, :], 0.0,
                    b1[:, :wg, :], mybir.AluOpType.is_gt,
                    mybir.AluOpType.subtract)
            k0 += wg

        # x: stream per token-tile (one contiguous 1 MB read), sign ->
        # mm_dt, matmul, epilogue, store
        for t in range(TT):
            if t == 0:
                xb = xb0
            else:
                xstg = xstage.tile([P, KT, P], bf16, tag="xstg")
                nc.sync.dma_start(out=xstg, in_=xt[t])
                xb = xpool.tile([P, KT, P], mm_dt, tag="xb")
                nc.scalar.sign(xb, xstg)
            ps = pspool.tile([P, dout], f32)
            if use_fp8:
                # DoubleRow: two K-subtiles per pass (K=256)
                for kp in range(KT // 2):
                    for b2 in range(NB):
                        nc.tensor.matmul(
                            ps[:, ts(b2, 512)],
                            lhsT=xb[:, 2 * kp:2 * kp + 2, :],
                            rhs=wsb[:, 2 * kp:2 * kp + 2, ts(b2, 512)],
                            start=(kp == 0),
                            stop=(kp == KT // 2 - 1),
                            perf_mode=mybir.MatmulPerfMode.DoubleRow,
                        )
            else:
                for k in range(KT):
                    for b2 in range(NB):
                        nc.tensor.matmul(
                            ps[:, ts(b2, 512)],
                            lhsT=xb[:, k, :],
                            rhs=wsb[:, k, ts(b2, 512)],
                            start=(k == 0),
                            stop=(k == KT - 1),
                        )
            osb = opool.tile([P, dout], f32)
            nc.vector.tensor_mul(osb, ps, alphaB)
            nc.vector.tensor_add(osb, osb, biasB)
            nc.sync.dma_start(out=out[ts(t, P), :], in_=osb)
    nc.finalize()
    return nc


def _shard_inputs(x, weight, bias, alpha):
    import ml_dtypes

    bf16 = ml_dtypes.bfloat16
    P = 128
    KT = DIN // P
    TT = TOK // P

    x2 = np.asarray(x, dtype=np.float32).reshape(NTOK, DIN)
    w = np.asarray(weight, dtype=np.float32)
    bias = np.asarray(bias, dtype=np.float32).reshape(-1)
    alpha_f = np.asarray(alpha, dtype=np.float32).reshape(-1)

    xTs = []
    for r in range(R):
        xT = x2[r * TOK:(r + 1) * TOK, :].T.astype(bf16)  # [DIN, TOK]
        # -> [TT, 128(p), KT, 128(t)]: each [p, kt, t] token-tile contiguous
        xt_tiled = np.ascontiguousarray(
            xT.reshape(KT, P, TT, P).transpose(2, 1, 0, 3))
        xTs.append(xt_tiled)
    wT = w.T.astype(bf16)  # [DIN, DOUT_FULL]
    wTs = [np.ascontiguousarray(wT[:, c * DOUT:(c + 1) * DOUT]) for c in range(C)]
    als = [np.ascontiguousarray(alpha_f[c * DOUT:(c + 1) * DOUT]) for c in range(C)]
    bis = [np.ascontiguousarray(bias[c * DOUT:(c + 1) * DOUT]) for c in range(C)]

    in_maps = []
    for i in range(8):
        r, c = divmod(i, C)
        in_maps.append({"xt": xTs[r], "wt": wTs[c], "al": als[c], "bi": bis[c]})
    return in_maps


def kernel(x, weight, bias, alpha, _trace=False, _trace_cores=None):
    global LAST_RESULTS
    from concourse.bass_utils import run_bass_kernel_spmd

    in_maps = _shard_inputs(x, weight, bias, alpha)
    nc = build_nc()
    kwargs = {}
    if _trace:
        kwargs = dict(trace=True, trace_cores=_trace_cores or [0])
    res = run_bass_kernel_spmd(nc, in_maps, core_ids=list(range(8)), **kwargs)
    LAST_RESULTS = res

    out = np.empty((NTOK, DOUT_FULL), dtype=np.float32)
    for i in range(8):
        r, c = divmod(i, C)
        out[r * TOK:(r + 1) * TOK, c * DOUT:(c + 1) * DOUT] = res.results[i]["out"]
    return out.reshape(B, S, DOUT_FULL)
